# revision 15
# baseline (speedup 1.0000x reference)
"""Trainium2 Bass kernel for MoRAttention (sparse selective-KV GQA attention).

Math note: the reference's argsort/gather of active keys is dense attention
over the gathered active keys with mask = pos[k] <= pos[q]; softmax +
weighted-sum are permutation invariant along the key axis and padded slots
contribute exp(-inf) = 0. The host gathers active columns of x per batch, so
k/v projection and attention run over skv ~ n_active keys instead of S.

Sharding: 8 cores = 2 batches x 4 kv-groups. Core (b, g) computes q-heads
[4g, 4g+4) and kv-head g of batch b, producing a partial o_proj output
[S, D]; the host sums the 4 partials per batch (all-reduce after o_proj).

Device layout (per core, matmul operands bf16):
  xT  [D, S]    full hidden (for q proj);  xTs [D, skv] gathered (for k/v)
  scores^T[k, q] = kTs_chunk^T.T @ qT   (k = partition axis)
  p = exp(scale * s^T + abias_k)        (abias kills padded keys)
  causal mask = compiled per-chunk column ranges [qst, S) + an elementwise
  "band" mask on columns [qst, qfull) where visibility is data-dependent
  colsum_bcast[:, q] = ones128^T @ p    (PE partition-reduce, broadcast)
  attnT[d, q] += v_chunk[k, d].T @ p
  attn_norm = attnT * recip(colsum);  out[q, D] += attnT_h[:, qtile].T @ wo_h

Scheduling notes (from NTFF traces):
 - PE p-state ramps to 2.4 GHz only under continuous execution => phase A
   is six single-ftile passes (k, v, h0..h3); pass i's rope eviction hides
   under pass i+1's matmuls.
 - Phase B software-pipelines: attn@v/colsum of chunk kc-1 are emitted
   after the scores of chunk kc, so the PE streams through exp latency.
 - start=True resets the whole PSUM bank => accumulation regions are
   bank-aligned (512 f32 cols).
 - e/rb/out tiles live in dedicated SBUF (no pool recycling WARs).
 - wo/band loads are emitted after phase A so their DMA doesn't steal HBM
   bandwidth from the critical early xT/w loads.
"""

import numpy as np

S, D, HD = 1024, 2048, 128
NH = 4          # q heads per core
DC = D // 128   # D chunks
SCALE = HD ** -0.5
NEG = -30.0     # additive logit bias for padded keys (exp -> ~1e-13)

TRACE = False
LAST_EXEC_NS = None
LAST_RESULTS = None

_NC_CACHE = {}


def _build_nc(skv, qst, qfull):
    import concourse.bass as bass
    import concourse.mybir as mybir
    from concourse import bacc
    from concourse.tile import TileContext
    from concourse.masks import make_identity
    from contextlib import ExitStack

    f32 = mybir.dt.float32
    bf16 = mybir.dt.bfloat16
    Exp = mybir.ActivationFunctionType.Exp

    KCS = skv // 128
    bw = [qfull[kc] - qst[kc] for kc in range(KCS)]
    boff = np.concatenate([[0], np.cumsum(bw)]).astype(int)
    BW = int(boff[-1])
    r0_last = max(kc for kc in range(KCS) if qst[kc] < 512)

    nc = bacc.Bacc("TRN2", target_bir_lowering=False, debug=False)

    xT_d = nc.dram_tensor("xT", [D, S], bf16, kind="ExternalInput")
    # xTs ships pre-laid-out as the SBUF image [128, DC*skv] so each DMA
    # moves >=2KB-per-partition lines (skv-wide chunks alone are only
    # skv*2 bytes per line, which halves DMA efficiency)
    xTs_d = nc.dram_tensor("xTs", [128, DC * skv], bf16, kind="ExternalInput")
    wq_d = nc.dram_tensor("wqs", [D, NH * HD], bf16, kind="ExternalInput")
    wk_d = nc.dram_tensor("wks", [D, HD], bf16, kind="ExternalInput")
    wv_d = nc.dram_tensor("wvs", [D, HD], bf16, kind="ExternalInput")
    wo_d = nc.dram_tensor("wos", [NH * HD, D], bf16, kind="ExternalInput")
    cos_d = nc.dram_tensor("cosT", [HD, S], f32, kind="ExternalInput")
    sinr_d = nc.dram_tensor("sinrT", [HD, S], f32, kind="ExternalInput")
    coss_d = nc.dram_tensor("cosTs", [HD, skv], f32, kind="ExternalInput")
    sinrs_d = nc.dram_tensor("sinrTs", [HD, skv], f32, kind="ExternalInput")
    abias_d = nc.dram_tensor("abias", [128, KCS], f32, kind="ExternalInput")
    band_d = nc.dram_tensor("band", [128, BW], bf16, kind="ExternalInput")
    out_d = nc.dram_tensor("out", [S, D], bf16, kind="ExternalOutput")

    with TileContext(nc) as tc, ExitStack() as ctx:
        singles = ctx.enter_context(tc.tile_pool(name="singles", bufs=1))
        persist = ctx.enter_context(tc.tile_pool(name="persist", bufs=1))

        identity = singles.tile([128, 128], bf16)
        make_identity(nc, identity)
        ones128 = singles.tile([128, 128], bf16)
        nc.vector.memset(ones128, 1.0)

        # small / rope inputs on the scalar-engine DGE queue (sync queue is
        # dedicated to the phase-A-critical x/w loads)
        abias = singles.tile([128, KCS], f32)
        nc.scalar.dma_start(out=abias, in_=abias_d[:, :])
        coss_sb = singles.tile([128, skv], f32)
        nc.scalar.dma_start(out=coss_sb, in_=coss_d[:, :])
        sinrs_sb = singles.tile([128, skv], f32)
        nc.scalar.dma_start(out=sinrs_sb, in_=sinrs_d[:, :])
        cos_sb = singles.tile([128, S], f32)
        nc.scalar.dma_start(out=cos_sb, in_=cos_d[:, :])
        sinr_sb = singles.tile([128, S], f32)
        nc.scalar.dma_start(out=sinr_sb, in_=sinr_d[:, :])
        band_sb = singles.tile([128, max(BW, 1)], bf16)
        nc.scalar.dma_start(out=band_sb[:, 0:BW], in_=band_d[:, :])

        # sync-queue loads in first-use order: pass order is k, v, h0..h3
        xTs_sb = persist.tile([128, DC * skv], bf16, tag="xTs_sb")
        xT_sb = persist.tile([128, DC * S], bf16, tag="xT_sb")
        wq_sb = persist.tile([128, DC * 512], bf16, tag="wq_sb")
        wk_sb = persist.tile([128, DC * 128], bf16, tag="wk_sb")
        wv_sb = persist.tile([128, DC * 128], bf16, tag="wv_sb")
        wo_sb = persist.tile([128, NH * D], bf16, tag="wo_sb")

        wq4 = wq_sb.rearrange("p (g c f) -> p g c f", g=4, c=4)
        wqd4 = wq_d.rearrange("(g c p) f -> p g c f", g=4, p=128)
        wk2 = wk_sb.rearrange("p (g c f) -> p g c f", g=2, c=8)
        wkd2 = wk_d.rearrange("(g c p) f -> p g c f", g=2, p=128)
        wv2 = wv_sb.rearrange("p (g c f) -> p g c f", g=2, c=8)
        wvd2 = wv_d.rearrange("(g c p) f -> p g c f", g=2, p=128)

        def ld_xts(c0, c1):
            nc.sync.dma_start(
                out=xTs_sb[:, c0 * skv:c1 * skv], in_=xTs_d[:, c0 * skv:c1 * skv]
            )

        def ld_xt(c0, c1):
            for c in range(c0, c1):
                nc.sync.dma_start(
                    out=xT_sb[:, c * S:(c + 1) * S], in_=xT_d[c * 128:(c + 1) * 128, :]
                )

        nc.sync.dma_start(out=wk2[:, 0], in_=wkd2[:, 0])
        ld_xts(0, 2)
        ld_xts(2, 4)
        nc.sync.dma_start(out=wv2[:, 0], in_=wvd2[:, 0])
        ld_xts(4, 6)
        ld_xts(6, 8)
        nc.sync.dma_start(out=wk2[:, 1], in_=wkd2[:, 1])
        nc.sync.dma_start(out=wv2[:, 1], in_=wvd2[:, 1])
        ld_xts(8, 12)
        ld_xts(12, DC)
        nc.sync.dma_start(out=wq4[:, 0], in_=wqd4[:, 0])
        ld_xt(0, 4)
        nc.sync.dma_start(out=wq4[:, 1], in_=wqd4[:, 1])
        ld_xt(4, 10)
        nc.sync.dma_start(out=wq4[:, 2], in_=wqd4[:, 2])
        ld_xt(10, DC)
        nc.sync.dma_start(out=wq4[:, 3], in_=wqd4[:, 3])

        qT = [persist.tile([128, S], bf16, tag=f"qT{h}", name=f"qT{h}") for h in range(NH)]
        kT = persist.tile([128, skv], bf16, tag="kT")
        vT = persist.tile([128, skv], bf16, tag="vT")
        vn = persist.tile([128, skv], bf16, tag="vn")  # v chunk kc in [k, hd]
        attn = [persist.tile([128, S], bf16, tag=f"attn{h}", name=f"attn{h}") for h in range(NH)]
        # dedicated phase-B/C tiles (never recycle rope space)
        e_t = [persist.tile([128, S], bf16, tag=f"e{kc}", name=f"e{kc}") for kc in range(KCS)]
        rb_t = [persist.tile([128, 512], f32, tag=f"rb{i}", name=f"rb{i}") for i in range(2)]
        out_t = [persist.tile([128, D], bf16, tag=f"ot{i}", name=f"ot{i}") for i in range(2)]

        # one-time zeros for e-columns below each chunk's causal start
        for kc in range(KCS):
            qlo = 0 if qst[kc] < 512 else 512
            if qst[kc] > qlo:
                nc.gpsimd.memset(e_t[kc][:, qlo:qst[kc]], 0.0)

        # ===== Phase A: projections =====
        with tc.tile_pool(name="ppsum", bufs=3, space="PSUM") as ppsum, \
             tc.tile_pool(name="ptrp", bufs=2, space="PSUM") as ptrp, \
             tc.tile_pool(name="rope", bufs=2) as rope_pool:

            def rope_evict(psum, dest, n, cos_t, sinr_t):
                # dest[:, :n] = psum*cos + rotate_half(psum)*sin (pre-signed)
                src = rope_pool.tile([128, S], f32, tag="ropesrc", name="ropesrc")
                nc.scalar.copy(src[:, 0:n], psum[:, 0:n])
                tmp = rope_pool.tile([128, S], f32, tag="ropetmp", name="ropetmp")
                nc.scalar.dma_start(out=tmp[0:64, 0:n], in_=src[64:128, 0:n])
                nc.scalar.dma_start(out=tmp[64:128, 0:n], in_=src[0:64, 0:n])
                nc.vector.tensor_mul(tmp[:, 0:n], tmp[:, 0:n], sinr_t)
                nc.vector.tensor_mul(src[:, 0:n], src[:, 0:n], cos_t)
                nc.vector.tensor_add(dest, src[:, 0:n], tmp[:, 0:n])

            for f in (4, 5, 0, 1, 2, 3):  # k, v, h0, h1, h2, h3
                n = skv if f >= 4 else S
                psum = ppsum.tile([128, S], f32, tag="pp", name=f"pp{f}")
                for c in range(DC):
                    if f < 4:
                        lhsT = wq_sb[:, c * 512 + f * 128: c * 512 + (f + 1) * 128]
                        rhs_t, rw = xT_sb, S
                    elif f == 4:
                        lhsT = wk_sb[:, c * 128:(c + 1) * 128]
                        rhs_t, rw = xTs_sb, skv
                    else:
                        lhsT = wv_sb[:, c * 128:(c + 1) * 128]
                        rhs_t, rw = xTs_sb, skv
                    for qs, qe in ((0, min(512, n)), (512, n)):
                        if qs >= qe:
                            continue
                        nc.tensor.matmul(
                            psum[:, qs:qe],
                            lhsT=lhsT,
                            rhs=rhs_t[:, c * rw + qs: c * rw + qe],
                            start=(c == 0), stop=(c == DC - 1),
                        )
                if f < 4:
                    rope_evict(psum, qT[f], S, cos_sb, sinr_sb)
                elif f == 4:
                    rope_evict(psum, kT, skv, coss_sb, sinrs_sb)
                else:
                    nc.scalar.copy(vT, psum[:, 0:skv])
                if f == 0:
                    # v: [HD, skv] -> [skv, HD] via PE transpose; vT was
                    # evicted during this pass, so no PE stall here
                    for kc in range(KCS):
                        pt = ptrp.tile([128, 128], bf16, tag="ptr")
                        nc.tensor.transpose(pt, vT[:, kc * 128:(kc + 1) * 128], identity)
                        nc.scalar.copy(vn[:, kc * 128:(kc + 1) * 128], pt)

        # wo rides the tail of the sync queue (transfers land well before
        # phase C; issuing on scalar would delay the first phase-B exp)
        for h in range(NH):
            nc.sync.dma_start(
                out=wo_sb[:, h * D:(h + 1) * D], in_=wo_d[h * 128:(h + 1) * 128, :]
            )

        # ===== Phase B: attention, head-sequential, software-pipelined =====
        # Flattened (h, kc) pipeline with lag-1 attn@v so the PE streams
        # through both exp latency and head boundaries: at a boundary,
        # scores/exp of (h+1, 0) are emitted before attn@v(h, last) and the
        # normalization of head h.
        with tc.tile_pool(name="ps", bufs=2, space="PSUM") as ps_p, \
             tc.tile_pool(name="po", bufs=1, space="PSUM") as po_p, \
             tc.tile_pool(name="pcb", bufs=1, space="PSUM") as pcb_p:
            po_t, pcb_t = {}, {}

            def emit_av(h, kc):
                # colsum(broadcast) and attn@v of chunk (h, kc)
                kcs = kc * 128
                for qs in ((0, 512) if qst[kc] < 512 else (512,)):
                    stop = kc == (r0_last if qs == 0 else KCS - 1)
                    nc.tensor.matmul(
                        pcb_t[h][:, qs:qs + 512],
                        lhsT=ones128,
                        rhs=e_t[kc][:, qs:qs + 512],
                        start=(kc == 0), stop=stop,
                    )
                    nc.tensor.matmul(
                        po_t[h][:, qs:qs + 512],
                        lhsT=vn[:, kcs:kcs + 128],
                        rhs=e_t[kc][:, qs:qs + 512],
                        start=(kc == 0), stop=stop,
                    )
                if kc == r0_last:
                    normalize(h, 0)
                if kc == KCS - 1:
                    if r0_last == KCS - 1:
                        normalize(h, 0)
                    normalize(h, 1)

            def normalize(h, i):
                # left half (i=0) is final after r0_last; right at head end
                sl = slice(512 * i, 512 * i + 512)
                nc.vector.reciprocal_approx_fast(rb_t[i], pcb_t[h][:, sl])
                nc.vector.tensor_mul(attn[h][:, sl], po_t[h][:, sl], rb_t[i])

            seq = [(h, kc) for h in range(NH) for kc in range(KCS)]
            for i, (h, kc) in enumerate(seq):
                if kc == 0:
                    po_t[h] = po_p.tile([128, S], f32, tag="po", name=f"po{h}")
                    pcb_t[h] = pcb_p.tile([128, S], f32, tag="pcb", name=f"pcb{h}")
                qa = qst[kc]
                psum_s = ps_p.tile([128, S], f32, tag="ps")
                regions = ((qa, 512), (512, S)) if qa < 512 else ((qa, S),)
                for qs, qe in regions:
                    if qs >= qe:
                        continue
                    nc.tensor.matmul(
                        psum_s[:, qs:qe],
                        lhsT=kT[:, kc * 128:(kc + 1) * 128],
                        rhs=qT[h][:, qs:qe],
                        start=True, stop=True,
                    )
                # exp(scale*scores + pad_bias[key]); padded keys -> ~0
                nc.scalar.activation(
                    e_t[kc][:, qa:S], psum_s[:, qa:S], Exp,
                    bias=abias[:, kc:kc + 1], scale=SCALE,
                )
                # data-dependent causal band on columns [qst, qfull)
                if bw[kc] > 0:
                    nc.vector.tensor_mul(
                        e_t[kc][:, qa:qfull[kc]],
                        e_t[kc][:, qa:qfull[kc]],
                        band_sb[:, boff[kc]:boff[kc + 1]],
                    )
                if i > 0:
                    emit_av(*seq[i - 1])
            emit_av(*seq[-1])

        # ===== Phase C: partial o_proj =====
        with tc.tile_pool(name="opsum", bufs=2, space="PSUM") as opsum:
            for qt in range(S // 128):
                ocs = [opsum.tile([128, S], f32, tag=f"oc{j}", name=f"oc{j}") for j in range(2)]
                for h in range(NH):
                    lhsT = attn[h][:, qt * 128:(qt + 1) * 128]
                    for j in range(4):
                        nc.tensor.matmul(
                            ocs[j // 2][:, (j % 2) * 512:(j % 2 + 1) * 512],
                            lhsT=lhsT,
                            rhs=wo_sb[:, h * D + j * 512: h * D + (j + 1) * 512],
                            start=(h == 0), stop=(h == NH - 1),
                        )
                outsb = out_t[qt % 2]
                nc.vector.tensor_copy(outsb[:, 0:S], ocs[0])
                nc.scalar.copy(outsb[:, S:D], ocs[1])
                eng = nc.sync if qt % 2 == 0 else nc.scalar
                eng.dma_start(out=out_d[qt * 128:(qt + 1) * 128, :], in_=outsb)

    nc.compile()
    return nc


def _get_nc(skv, qst, qfull):
    key = (skv, tuple(qst), tuple(qfull))
    if key not in _NC_CACHE:
        _NC_CACHE[key] = _build_nc(skv, qst, qfull)
    return _NC_CACHE[key]


def _host_prep(hidden_states, cos, sin, wq, wk, wv, wo, position_ids, active_mask):
    import ml_dtypes
    bf16 = ml_dtypes.bfloat16

    hs = np.asarray(hidden_states, dtype=np.float32)
    cos = np.asarray(cos, dtype=np.float32)
    sin = np.asarray(sin, dtype=np.float32)
    pos = np.asarray(position_ids)
    am = np.asarray(active_mask).astype(bool)
    B = hs.shape[0]

    assert B == 2 and hs.shape[1] == S and hs.shape[2] == D
    # the device schedule bakes pos == arange (what setup_inputs produces)
    assert np.array_equal(pos, np.tile(np.arange(S, dtype=pos.dtype), (B, 1)))

    cosT = np.ascontiguousarray(cos.T)               # [HD, S]
    sinT = sin.T
    sinrT = np.ascontiguousarray(np.concatenate([-sinT[:64], sinT[64:]], axis=0))

    # gather active keys (actives first, stable order = ascending position)
    n_act = [int(am[b].sum()) for b in range(B)]
    skv = max(128, -(-max(n_act) // 128) * 128)
    KCS = skv // 128
    idx = np.zeros((B, skv), np.int64)
    pos_sel = np.full((B, skv), 10 * S, np.int64)    # pad sentinel
    for b in range(B):
        a = np.where(am[b])[0]
        idx[b, :len(a)] = a
        pos_sel[b, :len(a)] = a

    # per-chunk causal schedule (union over batches)
    qst, qfull = [], []
    for kc in range(KCS):
        lo, hi = [], []
        for b in range(B):
            pp = pos_sel[b, kc * 128:(kc + 1) * 128]
            real = pp[pp < S]
            if len(real):
                lo.append(int(real.min())); hi.append(int(real.max()))
        qst.append(128 * (min(lo) // 128) if lo else S - 128)
        qfull.append(128 * (-(-(max(hi) + 1) // 128)) if hi else S)
    bw = [qfull[kc] - qst[kc] for kc in range(KCS)]
    boff = np.concatenate([[0], np.cumsum(bw)]).astype(int)
    BW = int(boff[-1])

    wqc = np.asarray(wq, dtype=np.float32).astype(bf16)
    wkc = np.asarray(wk, dtype=np.float32).astype(bf16)
    wvc = np.asarray(wv, dtype=np.float32).astype(bf16)
    woc = np.asarray(wo, dtype=np.float32).astype(bf16)

    in_maps = []
    for core in range(8):
        b, g = divmod(core, 4)
        pclip = np.minimum(pos_sel[b], S - 1)
        abias = np.where(pos_sel[b] < S, 0.0, NEG).astype(np.float32)
        abias = np.ascontiguousarray(abias.reshape(KCS, 128).T)   # [128, KCS]
        band = np.zeros((128, max(BW, 1)), np.float32)
        for kc in range(KCS):
            if bw[kc] > 0:
                qq = np.arange(qst[kc], qfull[kc])
                band[:, boff[kc]:boff[kc + 1]] = (
                    pos_sel[b, kc * 128:(kc + 1) * 128][:, None] <= qq[None, :]
                )
        xTb = np.ascontiguousarray(hs[b].T).astype(bf16)
        # xTs as the SBUF image [128, DC*skv] (chunk c at cols [c*skv, ...))
        xts = xTb[:, idx[b]].reshape(DC, 128, skv).transpose(1, 0, 2)
        in_maps.append({
            "xT": xTb,
            "xTs": np.ascontiguousarray(xts.reshape(128, DC * skv)),
            "wqs": np.ascontiguousarray(wqc[:, g * 512:(g + 1) * 512]),
            "wks": np.ascontiguousarray(wkc[:, g * 128:(g + 1) * 128]),
            "wvs": np.ascontiguousarray(wvc[:, g * 128:(g + 1) * 128]),
            "wos": np.ascontiguousarray(woc[g * 512:(g + 1) * 512, :]),
            "cosT": cosT,
            "sinrT": sinrT,
            "cosTs": np.ascontiguousarray(cosT[:, pclip]),
            "sinrTs": np.ascontiguousarray(sinrT[:, pclip]),
            "abias": abias,
            "band": band.astype(bf16),
        })
    return in_maps, skv, qst, qfull


def kernel(hidden_states, cos, sin, wq, wk, wv, wo, position_ids, active_mask):
    global LAST_EXEC_NS, LAST_RESULTS
    from concourse.bass_utils import run_bass_kernel_spmd

    in_maps, skv, qst, qfull = _host_prep(
        hidden_states, cos, sin, wq, wk, wv, wo, position_ids, active_mask
    )
    nc = _get_nc(skv, qst, qfull)
    res = run_bass_kernel_spmd(nc, in_maps, core_ids=list(range(8)), trace=TRACE)
    LAST_EXEC_NS = res.exec_time_ns
    LAST_RESULTS = res
    outs = [np.asarray(res.results[c]["out"], dtype=np.float32) for c in range(8)]
    B = np.asarray(hidden_states).shape[0]
    full = np.stack(
        [sum(outs[b * 4 + g] for g in range(4)) for b in range(B)], axis=0
    )
    return full.astype(np.float32)


# revision 25
# speedup vs baseline: 1.2006x; 1.2006x over previous
"""Trainium2 Bass kernel for MoRAttention (sparse selective-KV GQA attention).

Math note: the reference's argsort/gather of active keys is dense attention
over the gathered active keys with mask = pos[k] <= pos[q]; softmax +
weighted-sum are permutation invariant along the key axis and padded slots
contribute exp(-inf) = 0. The host gathers active columns of x per batch, so
k/v projection and attention run over skv ~ n_active keys instead of S.

Sharding: 8 cores = 2 batches x 4 kv-groups. Core (b, g) computes q-heads
[4g, 4g+4) and kv-head g of batch b, producing a partial o_proj output
[S, D]; the host sums the 4 partials per batch (all-reduce after o_proj).

Device layout (per core, matmul operands bf16):
  xT  [D, S]    full hidden (for q proj);  xTs [D, skv] gathered (for k/v)
  scores^T[k, q] = kTs_chunk^T.T @ qT   (k = partition axis)
  p = exp(scale * s^T + abias_k)        (abias kills padded keys)
  causal mask = compiled per-chunk column ranges [qst, S) + an elementwise
  "band" mask on columns [qst, qfull) where visibility is data-dependent
  colsum_bcast[:, q] = ones128^T @ p    (PE partition-reduce, broadcast)
  attnT[d, q] += v_chunk[k, d].T @ p
  attn_norm = attnT * recip(colsum);  out[q, D] += attnT_h[:, qtile].T @ wo_h

Scheduling notes (from NTFF traces):
 - PE p-state ramps to 2.4 GHz only under continuous execution => phase A
   is six single-ftile passes (k, v, h0..h3); pass i's rope eviction hides
   under pass i+1's matmuls.
 - Phase B software-pipelines: attn@v/colsum of chunk kc-1 are emitted
   after the scores of chunk kc, so the PE streams through exp latency.
 - start=True resets the whole PSUM bank => accumulation regions are
   bank-aligned (512 f32 cols).
 - e/rb/out tiles live in dedicated SBUF (no pool recycling WARs).
 - wo/band loads are emitted after phase A so their DMA doesn't steal HBM
   bandwidth from the critical early xT/w loads.
"""

import numpy as np

S, D, HD = 1024, 2048, 128
NH = 4          # q heads per core
DC = D // 128   # D chunks
SCALE = HD ** -0.5
NEG = -30.0     # additive logit bias for padded keys (exp -> ~1e-13)

TRACE = False
LAST_EXEC_NS = None
LAST_RESULTS = None

_NC_CACHE = {}


def _build_nc(skv, qst, qfull):
    import concourse.bass as bass
    import concourse.mybir as mybir
    from concourse import bacc
    from concourse.tile import TileContext
    from concourse.masks import make_identity
    from contextlib import ExitStack

    f32 = mybir.dt.float32
    bf16 = mybir.dt.bfloat16
    Exp = mybir.ActivationFunctionType.Exp

    KCS = skv // 128
    bw = [qfull[kc] - qst[kc] for kc in range(KCS)]
    boff = np.concatenate([[0], np.cumsum(bw)]).astype(int)
    BW = int(boff[-1])
    r0_last = max(kc for kc in range(KCS) if qst[kc] < 512)

    nc = bacc.Bacc("TRN2", target_bir_lowering=False, debug=False)

    xT_d = nc.dram_tensor("xT", [D, S], bf16, kind="ExternalInput")
    # xTs ships pre-laid-out as the SBUF image [128, DC*skv] so each DMA
    # moves >=2KB-per-partition lines (skv-wide chunks alone are only
    # skv*2 bytes per line, which halves DMA efficiency)
    xTs_d = nc.dram_tensor("xTs", [128, DC * skv], bf16, kind="ExternalInput")
    wq_d = nc.dram_tensor("wqs", [D, NH * HD], bf16, kind="ExternalInput")
    wk_d = nc.dram_tensor("wks", [D, HD], bf16, kind="ExternalInput")
    wv_d = nc.dram_tensor("wvs", [D, HD], bf16, kind="ExternalInput")
    wo_d = nc.dram_tensor("wos", [NH * HD, D], bf16, kind="ExternalInput")
    cos_d = nc.dram_tensor("cosT", [HD, S], f32, kind="ExternalInput")
    sinr_d = nc.dram_tensor("sinrT", [HD, S], f32, kind="ExternalInput")
    coss_d = nc.dram_tensor("cosTs", [HD, skv], f32, kind="ExternalInput")
    sinrs_d = nc.dram_tensor("sinrTs", [HD, skv], f32, kind="ExternalInput")
    abias_d = nc.dram_tensor("abias", [128, KCS], f32, kind="ExternalInput")
    band_d = nc.dram_tensor("band", [128, BW], bf16, kind="ExternalInput")
    out_d = nc.dram_tensor("out", [S, D], bf16, kind="ExternalOutput")

    with TileContext(nc) as tc, ExitStack() as ctx:
        singles = ctx.enter_context(tc.tile_pool(name="singles", bufs=1))
        persist = ctx.enter_context(tc.tile_pool(name="persist", bufs=1))

        identity = singles.tile([128, 128], bf16)
        make_identity(nc, identity)
        ones128 = singles.tile([128, 128], bf16)
        nc.vector.memset(ones128, 1.0)

        # small / rope inputs on the scalar-engine DGE queue (sync queue is
        # dedicated to the phase-A-critical x/w loads)
        abias = singles.tile([128, KCS], f32)
        nc.scalar.dma_start(out=abias, in_=abias_d[:, :])
        coss_sb = singles.tile([128, skv], f32)
        nc.scalar.dma_start(out=coss_sb, in_=coss_d[:, :])
        sinrs_sb = singles.tile([128, skv], f32)
        nc.scalar.dma_start(out=sinrs_sb, in_=sinrs_d[:, :])
        cos_sb = singles.tile([128, S], f32)
        nc.scalar.dma_start(out=cos_sb, in_=cos_d[:, :])
        sinr_sb = singles.tile([128, S], f32)
        nc.scalar.dma_start(out=sinr_sb, in_=sinr_d[:, :])
        band_sb = singles.tile([128, max(BW, 1)], bf16)
        nc.scalar.dma_start(out=band_sb[:, 0:BW], in_=band_d[:, :])

        # sync-queue loads in first-use order: pass order is k, v, h0..h3
        xTs_sb = persist.tile([128, DC * skv], bf16, tag="xTs_sb")
        xT_sb = persist.tile([128, DC * S], bf16, tag="xT_sb")
        wq_sb = persist.tile([128, DC * 512], bf16, tag="wq_sb")
        wk_sb = persist.tile([128, DC * 128], bf16, tag="wk_sb")
        wv_sb = persist.tile([128, DC * 128], bf16, tag="wv_sb")
        wo_sb = persist.tile([128, NH * D], bf16, tag="wo_sb")

        wq4 = wq_sb.rearrange("p (g c f) -> p g c f", g=4, c=4)
        wqd4 = wq_d.rearrange("(g c p) f -> p g c f", g=4, p=128)
        wk4 = wk_sb.rearrange("p (g c f) -> p g c f", g=4, c=4)
        wkd4 = wk_d.rearrange("(g c p) f -> p g c f", g=4, p=128)
        wv2 = wv_sb.rearrange("p (g c f) -> p g c f", g=2, c=8)
        wvd2 = wv_d.rearrange("(g c p) f -> p g c f", g=2, p=128)

        def ld_xts(c0, c1):
            nc.sync.dma_start(
                out=xTs_sb[:, c0 * skv:c1 * skv], in_=xTs_d[:, c0 * skv:c1 * skv]
            )

        def ld_xt(c0, c1):
            for c in range(c0, c1):
                nc.sync.dma_start(
                    out=xT_sb[:, c * S:(c + 1) * S], in_=xT_d[c * 128:(c + 1) * 128, :]
                )

        nc.sync.dma_start(out=wk4[:, 0], in_=wkd4[:, 0])
        ld_xts(0, 2)
        nc.sync.dma_start(out=wk4[:, 1], in_=wkd4[:, 1])
        ld_xts(2, 4)
        nc.sync.dma_start(out=wv2[:, 0], in_=wvd2[:, 0])
        ld_xts(4, 6)
        nc.sync.dma_start(out=wk4[:, 2], in_=wkd4[:, 2])
        ld_xts(6, 8)
        nc.sync.dma_start(out=wk4[:, 3], in_=wkd4[:, 3])
        nc.sync.dma_start(out=wv2[:, 1], in_=wvd2[:, 1])
        ld_xts(8, 12)
        ld_xts(12, DC)
        nc.sync.dma_start(out=wq4[:, 0], in_=wqd4[:, 0])
        ld_xt(0, 4)
        nc.sync.dma_start(out=wq4[:, 1], in_=wqd4[:, 1])
        ld_xt(4, 10)
        nc.sync.dma_start(out=wq4[:, 2], in_=wqd4[:, 2])
        ld_xt(10, DC)
        nc.sync.dma_start(out=wq4[:, 3], in_=wqd4[:, 3])

        qT = [persist.tile([128, S], bf16, tag=f"qT{h}", name=f"qT{h}") for h in range(NH)]
        kT = persist.tile([128, skv], bf16, tag="kT")
        vT = persist.tile([128, skv], bf16, tag="vT")
        vn = persist.tile([128, skv], bf16, tag="vn")  # v chunk kc in [k, hd]
        attn = [persist.tile([128, S], bf16, tag=f"attn{h}", name=f"attn{h}") for h in range(NH)]
        # dedicated phase-B/C tiles (never recycle rope space)
        e_t = [persist.tile([128, S], bf16, tag=f"e{kc}", name=f"e{kc}") for kc in range(KCS)]
        rb_t = [persist.tile([128, 512], f32, tag=f"rb{i}", name=f"rb{i}") for i in range(2)]
        out_t = [persist.tile([128, D], bf16, tag=f"ot{i}", name=f"ot{i}") for i in range(2)]

        # one-time zeros for e-columns below each chunk's causal start
        for kc in range(KCS):
            qlo = 0 if qst[kc] < 512 else 512
            if qst[kc] > qlo:
                nc.gpsimd.memset(e_t[kc][:, qlo:qst[kc]], 0.0)

        # ===== Phase A: projections =====
        with tc.tile_pool(name="ppsum", bufs=3, space="PSUM") as ppsum, \
             tc.tile_pool(name="ptrp", bufs=2, space="PSUM") as ptrp, \
             tc.tile_pool(name="rope", bufs=2) as rope_pool:

            def rope_evict(psum, dest, n, cos_t, sinr_t, eng=None):
                # dest[:, :n] = psum*cos + rotate_half(psum)*sin (pre-signed)
                eng = eng or nc.vector
                src = rope_pool.tile([128, S], f32, tag="ropesrc", name="ropesrc")
                nc.scalar.copy(src[:, 0:n], psum[:, 0:n])
                tmp = rope_pool.tile([128, S], f32, tag="ropetmp", name="ropetmp")
                # half-swap on the scalar DGE queue: the sync queue still
                # holds multi-MB input loads, which would delay the swap
                nc.scalar.dma_start(out=tmp[0:64, 0:n], in_=src[64:128, 0:n])
                nc.scalar.dma_start(out=tmp[64:128, 0:n], in_=src[0:64, 0:n])
                eng.tensor_mul(tmp[:, 0:n], tmp[:, 0:n], sinr_t)
                eng.tensor_mul(src[:, 0:n], src[:, 0:n], cos_t)
                eng.tensor_add(dest, src[:, 0:n], tmp[:, 0:n])

            # h0 last: phase B starts with head 1, so h0's rope is the only
            # one that must hide under phase B (the rest hide under phase A)
            for f in (4, 5, 1, 2, 3, 0):  # k, v, h1, h2, h3, h0
                n = skv if f >= 4 else S
                psum = ppsum.tile([128, S], f32, tag="pp", name=f"pp{f}")
                for c in range(DC):
                    if f < 4:
                        lhsT = wq_sb[:, c * 512 + f * 128: c * 512 + (f + 1) * 128]
                        rhs_t, rw = xT_sb, S
                    elif f == 4:
                        lhsT = wk_sb[:, c * 128:(c + 1) * 128]
                        rhs_t, rw = xTs_sb, skv
                    else:
                        lhsT = wv_sb[:, c * 128:(c + 1) * 128]
                        rhs_t, rw = xTs_sb, skv
                    for qs, qe in ((0, min(512, n)), (512, n)):
                        if qs >= qe:
                            continue
                        nc.tensor.matmul(
                            psum[:, qs:qe],
                            lhsT=lhsT,
                            rhs=rhs_t[:, c * rw + qs: c * rw + qe],
                            start=(c == 0), stop=(c == DC - 1),
                        )
                if f < 4:
                    rope_evict(psum, qT[f], S, cos_sb, sinr_sb)
                elif f == 4:
                    rope_evict(psum, kT, skv, coss_sb, sinrs_sb)
                else:
                    nc.scalar.copy(vT, psum[:, 0:skv])
                if f == 1:
                    # v: [HD, skv] -> [skv, HD] via PE transpose; vT was
                    # evicted during this (third) pass, so no PE stall here
                    for kc in range(KCS):
                        pt = ptrp.tile([128, 128], bf16, tag="ptr")
                        nc.tensor.transpose(pt, vT[:, kc * 128:(kc + 1) * 128], identity)
                        nc.scalar.copy(vn[:, kc * 128:(kc + 1) * 128], pt)

        # wo rides the tail of the sync queue (transfers land well before
        # phase C; issuing on scalar would delay the first phase-B exp)
        for h in range(NH):
            nc.sync.dma_start(
                out=wo_sb[:, h * D:(h + 1) * D], in_=wo_d[h * 128:(h + 1) * 128, :]
            )

        # ===== Phase B: attention, head-sequential, software-pipelined =====
        # Flattened (h, kc) pipeline with lag-1 attn@v so the PE streams
        # through both exp latency and head boundaries: at a boundary,
        # scores/exp of (h+1, 0) are emitted before attn@v(h, last) and the
        # normalization of head h.
        with tc.tile_pool(name="ps", bufs=2, space="PSUM") as ps_p, \
             tc.tile_pool(name="po", bufs=1, space="PSUM") as po_p, \
             tc.tile_pool(name="pcb", bufs=1, space="PSUM") as pcb_p:
            po_t, pcb_t = {}, {}

            def emit_av(h, kc):
                # colsum(broadcast) and attn@v of chunk (h, kc)
                kcs = kc * 128
                for qs in ((0, 512) if qst[kc] < 512 else (512,)):
                    stop = kc == (r0_last if qs == 0 else KCS - 1)
                    nc.tensor.matmul(
                        pcb_t[h][:, qs:qs + 512],
                        lhsT=ones128,
                        rhs=e_t[kc][:, qs:qs + 512],
                        start=(kc == 0), stop=stop,
                    )
                    nc.tensor.matmul(
                        po_t[h][:, qs:qs + 512],
                        lhsT=vn[:, kcs:kcs + 128],
                        rhs=e_t[kc][:, qs:qs + 512],
                        start=(kc == 0), stop=stop,
                    )
                if kc == r0_last:
                    normalize(h, 0)
                if kc == KCS - 1:
                    if r0_last == KCS - 1:
                        normalize(h, 0)
                    normalize(h, 1)

            def normalize(h, i):
                # left half (i=0) is final after r0_last; right at head end
                sl = slice(512 * i, 512 * i + 512)
                nc.vector.reciprocal_approx_fast(rb_t[i], pcb_t[h][:, sl])
                nc.vector.tensor_mul(attn[h][:, sl], po_t[h][:, sl], rb_t[i])

            seq = [(h, kc) for h in (1, 2, 3, 0) for kc in range(KCS)]
            for i, (h, kc) in enumerate(seq):
                if kc == 0:
                    po_t[h] = po_p.tile([128, S], f32, tag="po", name=f"po{h}")
                    pcb_t[h] = pcb_p.tile([128, S], f32, tag="pcb", name=f"pcb{h}")
                qa = qst[kc]
                psum_s = ps_p.tile([128, S], f32, tag="ps")
                regions = ((qa, 512), (512, S)) if qa < 512 else ((qa, S),)
                for qs, qe in regions:
                    if qs >= qe:
                        continue
                    nc.tensor.matmul(
                        psum_s[:, qs:qe],
                        lhsT=kT[:, kc * 128:(kc + 1) * 128],
                        rhs=qT[h][:, qs:qe],
                        start=True, stop=True,
                    )
                # exp(scale*scores + pad_bias[key]); padded keys -> ~0
                nc.scalar.activation(
                    e_t[kc][:, qa:S], psum_s[:, qa:S], Exp,
                    bias=abias[:, kc:kc + 1], scale=SCALE,
                )
                # data-dependent causal band on columns [qst, qfull)
                if bw[kc] > 0:
                    nc.vector.tensor_mul(
                        e_t[kc][:, qa:qfull[kc]],
                        e_t[kc][:, qa:qfull[kc]],
                        band_sb[:, boff[kc]:boff[kc + 1]],
                    )
                if i > 0:
                    emit_av(*seq[i - 1])
            emit_av(*seq[-1])

        # ===== Phase C: partial o_proj =====
        with tc.tile_pool(name="opsum", bufs=2, space="PSUM") as opsum:
            for qt in range(S // 128):
                ocs = [opsum.tile([128, S], f32, tag=f"oc{j}", name=f"oc{j}") for j in range(2)]
                for h in range(NH):
                    lhsT = attn[h][:, qt * 128:(qt + 1) * 128]
                    for j in range(4):
                        nc.tensor.matmul(
                            ocs[j // 2][:, (j % 2) * 512:(j % 2 + 1) * 512],
                            lhsT=lhsT,
                            rhs=wo_sb[:, h * D + j * 512: h * D + (j + 1) * 512],
                            start=(h == 0), stop=(h == NH - 1),
                        )
                outsb = out_t[qt % 2]
                nc.vector.tensor_copy(outsb[:, 0:S], ocs[0])
                nc.scalar.copy(outsb[:, S:D], ocs[1])
                eng = nc.sync if qt % 2 == 0 else nc.scalar
                eng.dma_start(out=out_d[qt * 128:(qt + 1) * 128, :], in_=outsb)

    nc.compile()
    return nc


def _get_nc(skv, qst, qfull):
    key = (skv, tuple(qst), tuple(qfull))
    if key not in _NC_CACHE:
        _NC_CACHE[key] = _build_nc(skv, qst, qfull)
    return _NC_CACHE[key]


def _host_prep(hidden_states, cos, sin, wq, wk, wv, wo, position_ids, active_mask):
    import ml_dtypes
    bf16 = ml_dtypes.bfloat16

    hs = np.asarray(hidden_states, dtype=np.float32)
    cos = np.asarray(cos, dtype=np.float32)
    sin = np.asarray(sin, dtype=np.float32)
    pos = np.asarray(position_ids)
    am = np.asarray(active_mask).astype(bool)
    B = hs.shape[0]

    assert B == 2 and hs.shape[1] == S and hs.shape[2] == D
    # the device schedule bakes pos == arange (what setup_inputs produces)
    assert np.array_equal(pos, np.tile(np.arange(S, dtype=pos.dtype), (B, 1)))

    cosT = np.ascontiguousarray(cos.T)               # [HD, S]
    sinT = sin.T
    sinrT = np.ascontiguousarray(np.concatenate([-sinT[:64], sinT[64:]], axis=0))

    # gather active keys (actives first, stable order = ascending position)
    n_act = [int(am[b].sum()) for b in range(B)]
    skv = max(128, -(-max(n_act) // 128) * 128)
    KCS = skv // 128
    idx = np.zeros((B, skv), np.int64)
    pos_sel = np.full((B, skv), 10 * S, np.int64)    # pad sentinel
    for b in range(B):
        a = np.where(am[b])[0]
        idx[b, :len(a)] = a
        pos_sel[b, :len(a)] = a

    # per-chunk causal schedule (union over batches)
    qst, qfull = [], []
    for kc in range(KCS):
        lo, hi = [], []
        for b in range(B):
            pp = pos_sel[b, kc * 128:(kc + 1) * 128]
            real = pp[pp < S]
            if len(real):
                lo.append(int(real.min())); hi.append(int(real.max()))
        qst.append(128 * (min(lo) // 128) if lo else S - 128)
        qfull.append(128 * (-(-(max(hi) + 1) // 128)) if hi else S)
    bw = [qfull[kc] - qst[kc] for kc in range(KCS)]
    boff = np.concatenate([[0], np.cumsum(bw)]).astype(int)
    BW = int(boff[-1])

    wqc = np.asarray(wq, dtype=np.float32).astype(bf16)
    wkc = np.asarray(wk, dtype=np.float32).astype(bf16)
    wvc = np.asarray(wv, dtype=np.float32).astype(bf16)
    woc = np.asarray(wo, dtype=np.float32).astype(bf16)

    in_maps = []
    for core in range(8):
        b, g = divmod(core, 4)
        pclip = np.minimum(pos_sel[b], S - 1)
        abias = np.where(pos_sel[b] < S, 0.0, NEG).astype(np.float32)
        abias = np.ascontiguousarray(abias.reshape(KCS, 128).T)   # [128, KCS]
        band = np.zeros((128, max(BW, 1)), np.float32)
        for kc in range(KCS):
            if bw[kc] > 0:
                qq = np.arange(qst[kc], qfull[kc])
                band[:, boff[kc]:boff[kc + 1]] = (
                    pos_sel[b, kc * 128:(kc + 1) * 128][:, None] <= qq[None, :]
                )
        xTb = np.ascontiguousarray(hs[b].T).astype(bf16)
        # xTs as the SBUF image [128, DC*skv] (chunk c at cols [c*skv, ...))
        xts = xTb[:, idx[b]].reshape(DC, 128, skv).transpose(1, 0, 2)
        in_maps.append({
            "xT": xTb,
            "xTs": np.ascontiguousarray(xts.reshape(128, DC * skv)),
            "wqs": np.ascontiguousarray(wqc[:, g * 512:(g + 1) * 512]),
            "wks": np.ascontiguousarray(wkc[:, g * 128:(g + 1) * 128]),
            "wvs": np.ascontiguousarray(wvc[:, g * 128:(g + 1) * 128]),
            "wos": np.ascontiguousarray(woc[g * 512:(g + 1) * 512, :]),
            "cosT": cosT,
            "sinrT": sinrT,
            "cosTs": np.ascontiguousarray(cosT[:, pclip]),
            "sinrTs": np.ascontiguousarray(sinrT[:, pclip]),
            "abias": abias,
            "band": band.astype(bf16),
        })
    return in_maps, skv, qst, qfull


def kernel(hidden_states, cos, sin, wq, wk, wv, wo, position_ids, active_mask):
    global LAST_EXEC_NS, LAST_RESULTS
    from concourse.bass_utils import run_bass_kernel_spmd

    in_maps, skv, qst, qfull = _host_prep(
        hidden_states, cos, sin, wq, wk, wv, wo, position_ids, active_mask
    )
    nc = _get_nc(skv, qst, qfull)
    res = run_bass_kernel_spmd(nc, in_maps, core_ids=list(range(8)), trace=TRACE)
    LAST_EXEC_NS = res.exec_time_ns
    LAST_RESULTS = res
    outs = [np.asarray(res.results[c]["out"], dtype=np.float32) for c in range(8)]
    B = np.asarray(hidden_states).shape[0]
    full = np.stack(
        [sum(outs[b * 4 + g] for g in range(4)) for b in range(B)], axis=0
    )
    return full.astype(np.float32)
